# revision 34
# baseline (speedup 1.0000x reference)
"""Trainium2 Bass kernel for nn_Attention_12463995093474 (sparse_attention).

Math (reference):
  q/k/v = content linears; 2 absolute heads, 2 relative heads (DK=32).
  abs:  scores = (Xq_a + abs_kernel@abs_q_w) @ (Xk_a + abs_kernel@abs_k_w)^T
  rel:  scores = Xq_r @ Xk_r^T + (Xq_r + rel_bias) . (rel_kernel@rel_k_w + rel_k_b)
  softmax(mask) @ v -> out linear.

Key algebraic collapse: the dominant rel term
    s2[i,j] = sum_o q''[i,o] * (sum_d RK[i,j,d] W[d,o])  (+ c[i])
            = sum_d qW[i,d] * RK[i,j,d],   qW = q'' @ W^T
so RK contracts DIRECTLY against a per-row vector (32x fewer MACs than
materializing R).  Implementation:
  - rel_kernel is pre-transposed on host to [h, b, d, ipair, j, 2] fp8
    e4m3 (byte pair = rows 2m / 2m+1) so it streams with PLAIN DMAs and
    feeds fp8 dual-row matmuls.
  - per (head, row-PAIR) one PE matmul in MatmulPerfMode.DoubleRow:
    contract 256 = 2 k-tiles of 128 d, k-tile t = row 2m+t's RK; the
    stationary weights select each output row's own k-tile via a
    two-plane layout [qW[even rows] | qW[odd rows]] with zeros in the
    opposite plane.  The moving port runs at 2 B/partition/cycle --
    2x the fp8-1x rate (~43 ns per row of 200 scores).
  - dual-fp8 matmuls must write PSUM partition 0, so only the first
    32-row band of each row-block uses DoubleRow pairs; higher bands use
    fp8-1x per-row matmuls (plane-aware weight slices).  Rows descend
    within each band so start=True overwrites the garbage rows below.
    Content scores (with the c-term folded in as a 33rd contraction row
    of ones x c^T) accumulate on top, then masked softmax -> p@v.
  - qW is scaled by 64 on-device before the fp8 cast (values sigma
    ~0.005 would be subnormal in e4m3); the 2^-6 is folded into the
    epilogue's (s2 + c) * scale DVE op.

Sharding: data-parallel over batch, B=16 -> 2 batches per core on 8 cores.
"""

import numpy as np
from contextlib import ExitStack

import concourse.bass as bass
import concourse.bacc as bacc
import concourse.tile as tile
from concourse import mybir
from concourse.bass_utils import run_bass_kernel_spmd

B, T, D = 16, 200, 128
H_ABS, H_REL, H, DK = 2, 2, 4, 32
N_CORES = 8
BPC = B // N_CORES  # batches per core
SCALE = 1.0 / float(DK) ** 0.5
TT = BPC * T  # tokens per core (400)
UP = 64.0     # qW fp8 upscale (power of 2)

F32 = mybir.dt.float32
BF16 = mybir.dt.bfloat16
F8 = mybir.dt.float8e4
AX = mybir.AxisListType
OP = mybir.AluOpType
AF = mybir.ActivationFunctionType

# i-blocks per batch: (start, len)
IBLOCKS = [(0, 128), (128, T - 128)]


# packed weights, stored [128, col] on host; loaded with one plain DMA
WPACK_LAYOUT = [
    ("Wq", 128, 128), ("Wk", 128, 128), ("Wv", 128, 128), ("Wo", 128, 128),
    ("bq", 128, 1), ("bk", 128, 1),
    ("rkwT0", 32, 128), ("rkwT1", 32, 128),
    ("rkb0", 32, 1), ("rkb1", 32, 1), ("rbias0", 32, 1), ("rbias1", 32, 1),
    ("aqw0", 128, 32), ("aqw1", 128, 32), ("akw0", 128, 32), ("akw1", 128, 32),
    ("aqb0", 32, 1), ("aqb1", 32, 1), ("akb0", 32, 1), ("akb1", 32, 1),
    ("bqrb0", 32, 1), ("bqrb1", 32, 1),
    ("bvb", 128, 128), ("bob", 128, 128),
]
WPACK_OFF = {}
_o = 0
for _nm, _r, _cc in WPACK_LAYOUT:
    WPACK_OFF[_nm] = _o
    _o += _cc
WPACK_COLS = _o
# one host-packed [128, IN_COLS] bf16 tensor
IN_OFF_ID = WPACK_COLS
IN_OFF_Q = IN_OFF_ID + 128
IN_OFF_K = IN_OFF_Q + TT
IN_A_COLS = IN_OFF_K + TT          # part A: weights + ident + xq + xk
IN_OFF_V = IN_A_COLS
IN_OFF_A0 = IN_OFF_V + TT
IN_OFF_A1 = IN_OFF_A0 + TT
IN_OFF_M = IN_OFF_A1 + TT          # row-major mask
IN_COLS = IN_OFF_M + 4 * T


def build_kernel(ctx: ExitStack, tc: tile.TileContext, io: dict):
    nc = tc.nc

    relk = io["rel_kernel"]    # [2, 2, 128, 100, 200] bf16 carrier of fp8
    out = io["out"]            # [2, 200, 128]

    consts = ctx.enter_context(tc.tile_pool(name="consts", bufs=1))
    keep = ctx.enter_context(tc.tile_pool(name="keep", bufs=1))
    prep = ctx.enter_context(tc.tile_pool(name="prep", bufs=2))
    rkt_pool = ctx.enter_context(tc.tile_pool(name="rkt", bufs=7))
    sm = ctx.enter_context(tc.tile_pool(name="sm", bufs=2))
    # NOTE: this (otherwise unused) pool shifts downstream SBUF/semaphore
    # allocation in a way that measurably improves the schedule (~15us);
    # removing it regressed 105us -> 122us on hardware.
    smb = ctx.enter_context(tc.tile_pool(name="smb", bufs=3))
    ps_s2 = ctx.enter_context(tc.tile_pool(name="ps_s2", bufs=2, space="PSUM"))
    ps_pp = ctx.enter_context(tc.tile_pool(name="ps_pp", bufs=1, space="PSUM"))
    ps_x = ctx.enter_context(tc.tile_pool(name="ps_x", bufs=1, space="PSUM"))
    ps_ty = ctx.enter_context(tc.tile_pool(name="ps_ty", bufs=2, space="PSUM"))

    # ---------------- input load (plain DMAs only) ----------------
    # part A (weights+ident+xq+xk) lands first so prep unblocks early.
    inp = consts.tile([128, IN_COLS], BF16, tag="inpack")
    nc.sync.dma_start(inp[:, :IN_A_COLS], io["inpack"][:, :IN_A_COLS])
    nc.scalar.dma_start(inp[:, IN_A_COLS:], io["inpack"][:, IN_A_COLS:])

    def wslice(nm, rows=128):
        o = WPACK_OFF[nm]
        c = dict((n, cc) for n, _r, cc in WPACK_LAYOUT)[nm]
        return inp[:rows, o : o + c]

    xqT = inp[:, IN_OFF_Q : IN_OFF_Q + TT]
    xkT = inp[:, IN_OFF_K : IN_OFF_K + TT]
    xvT = inp[:, IN_OFF_V : IN_OFF_V + TT]
    akT_pre = {0: inp[:, IN_OFF_A0 : IN_OFF_A0 + TT],
               1: inp[:, IN_OFF_A1 : IN_OFF_A1 + TT]}
    ident = inp[:, IN_OFF_ID : IN_OFF_ID + 128]

    wq = wslice("Wq")
    wk = wslice("Wk")
    wv = wslice("Wv")
    wo = wslice("Wo")

    # ---- early consts: what gates the matvec streams + rel content ----
    bqrb64 = {}
    for hr in range(H_REL):
        t = consts.tile([DK, 1], F32, tag=f"bqrb64_{hr}")
        nc.scalar.activation(t, wslice(f"bqrb{hr}", rows=DK), AF.Copy,
                             scale=SCALE * UP)
        bqrb64[hr] = t
    bq_s64 = consts.tile([128, 1], F32, tag="bq_s64")
    nc.scalar.activation(bq_s64, wslice("bq"), AF.Copy, scale=SCALE * UP)
    bk_c = consts.tile([128, 1], F32, tag="bk_c")
    nc.scalar.copy(bk_c, wslice("bk"))

    qrb64 = {}   # rel heads q + rel_bias (x64), bf16 [32, TT]
    qT64 = {}    # rel content lhsT (x64), bf16 [32, TT]
    qW8 = {}     # two-plane DoubleRow weights, fp8 [128, 2*TT]
    kT = {}      # content k, bf16 [32, TT]

    def emit_rel_qw(hr):
        h = H_ABS + hr
        qp = ps_pp.tile([DK, TT], F32, tag="pp", name="pp")
        nc.tensor.matmul(qp, wq[:, DK * h : DK * (h + 1)], xqT,
                         skip_group_check=True)
        t2 = keep.tile([DK, TT], BF16, tag=f"qrb64_{hr}", name=f"qrb64_{hr}")
        nc.scalar.activation(t2, qp, AF.Identity, bias=bqrb64[hr],
                             scale=SCALE * UP)
        qrb64[hr] = t2
        qwp = ps_pp.tile([128, TT], F32, tag="pp", name="pp")
        nc.tensor.matmul(qwp, wslice(f"rkwT{hr}", rows=DK), t2,
                         skip_group_check=True)
        # DoubleRow weights, two-plane layout (ISA: ktile stride must be
        # even + 16B aligned; columns contiguous):
        #   plane0 (cols 0:TT):   qW[row] at even rows, 0 at odd
        #   plane1 (cols TT:2TT): 0 at even rows, qW[row] at odd
        q8 = keep.tile([128, 2 * TT], F8, tag=f"qW8_{hr}", name=f"qW8_{hr}")
        nc.vector.memset(q8, 0.0)
        rs_q = q8.ap[0][0]
        rs_p = qwp.ap[0][0]
        for par in range(2):
            dst = bass.AP(tensor=q8.tensor,
                          offset=q8.offset + TT * par + par,
                          ap=[[rs_q, 128], [2, TT // 2]])
            srcp = bass.AP(tensor=qwp.tensor, offset=qwp.offset + par,
                           ap=[[rs_p, 128], [2, TT // 2]])
            nc.vector.tensor_copy(dst, srcp)
        qW8[hr] = q8
        # rel content operands with the c-term folded in as an extra
        # contraction row: qT64c row 32 = c64^T, kTc row 32 = ones
        t = keep.tile([DK + 1, TT], BF16, tag=f"qT{h}", name=f"qT{h}")
        nc.scalar.activation(t[:DK, :], qp, AF.Identity,
                             bias=bq_s64[DK * h : DK * (h + 1)],
                             scale=SCALE * UP)
        cps = ps_pp.tile([64, TT], F32, tag="pp", name="pp")
        nc.tensor.matmul(cps[DK : DK + 1, :], wslice(f"rkb{hr}", rows=DK),
                         t2, skip_group_check=True,
                         tile_position=(0, DK))
        nc.scalar.copy(t[DK : DK + 1, :], cps[DK : DK + 1, :])
        qT64[hr] = t
        kp = ps_pp.tile([DK, TT], F32, tag="pp", name="pp")
        nc.tensor.matmul(kp, wk[:, DK * h : DK * (h + 1)], xkT,
                         skip_group_check=True)
        tk = keep.tile([DK + 1, TT], BF16, tag=f"kT{h}", name=f"kT{h}")
        nc.scalar.activation(tk[:DK, :], kp, AF.Identity,
                             bias=bk_c[DK * h : DK * (h + 1)])
        nc.vector.memset(tk[DK : DK + 1, :], 1.0)
        kT[h] = tk

    # ---- deferred prep (emitted under the first matvec streams) ----
    qT = {}
    vb = {}
    qaT = {}
    kaT = {}
    mb = {}
    c_sb = {}
    reph = {}

    def emit_rest_prep():
        bq_s = consts.tile([128, 1], F32, tag="bq_s")
        nc.scalar.activation(bq_s, wslice("bq"), AF.Copy, scale=SCALE)
        bv_b = consts.tile([128, 128], F32, tag="bv_b")
        nc.scalar.copy(bv_b, wslice("bvb"))
        bo_b = consts.tile([128, 128], F32, tag="bo_b")
        nc.scalar.copy(bo_b, wslice("bob"))
        reph["bo_b"] = bo_b

        small_cols = {}
        for hh in range(H_ABS):
            ts_ = consts.tile([DK, 1], F32, tag=f"aqb_s{hh}")
            nc.scalar.activation(ts_, wslice(f"aqb{hh}", rows=DK), AF.Copy,
                                 scale=SCALE)
            small_cols[("aqb_s", hh)] = ts_
            akb32 = consts.tile([DK, 1], F32, tag=f"akb32_{hh}")
            nc.scalar.copy(akb32, wslice(f"akb{hh}", rows=DK))
            small_cols[("akb", hh)] = akb32

        for h in range(H_ABS):
            qp = ps_pp.tile([DK, TT], F32, tag="pp", name="pp")
            nc.tensor.matmul(qp, wq[:, DK * h : DK * (h + 1)], xqT,
                             skip_group_check=True)
            t = keep.tile([DK, TT], BF16, tag=f"qT{h}", name=f"qT{h}")
            nc.scalar.activation(t, qp, AF.Identity,
                                 bias=bq_s[DK * h : DK * (h + 1)],
                                 scale=SCALE)
            qT[h] = t
            kp = ps_pp.tile([DK, TT], F32, tag="pp", name="pp")
            nc.tensor.matmul(kp, wk[:, DK * h : DK * (h + 1)], xkT,
                             skip_group_check=True)
            tk = keep.tile([DK, TT], BF16, tag=f"kT{h}", name=f"kT{h}")
            nc.scalar.activation(tk, kp, AF.Identity,
                                 bias=bk_c[DK * h : DK * (h + 1)])
            kT[h] = tk

        for b in range(BPC):
            for jb, (j0, jl) in enumerate(IBLOCKS):
                vp = ps_ty.tile([128, 128], F32, tag="ty", name="ty")
                nc.tensor.matmul(vp[:jl, :],
                                 xvT[:, b * T + j0 : b * T + j0 + jl], wv,
                                 skip_group_check=True)
                t = keep.tile([128, 128], BF16, tag=f"v{b}_{jb}",
                              name=f"v{b}_{jb}")
                nc.vector.tensor_add(t[:jl, :], vp[:jl, :], bv_b[:jl, :])
                vb[(b, jb)] = t

        for hh in range(H_ABS):
            akT = akT_pre[hh]
            pp = ps_pp.tile([DK, TT], F32, tag="pp", name="pp")
            nc.tensor.matmul(pp, wslice(f"aqw{hh}"), akT,
                             skip_group_check=True)
            pqT = prep.tile([DK, TT], BF16, tag="pqT", name="pqT")
            nc.scalar.activation(pqT, pp, AF.Identity,
                                 bias=small_cols[("aqb_s", hh)], scale=SCALE)
            t = keep.tile([DK, TT], BF16, tag=f"qaT{hh}", name=f"qaT{hh}")
            nc.vector.tensor_add(t, qT[hh], pqT)
            qaT[hh] = t

            pp2 = ps_pp.tile([DK, TT], F32, tag="pp", name="pp")
            nc.tensor.matmul(pp2, wslice(f"akw{hh}"), akT,
                             skip_group_check=True)
            pkT = prep.tile([DK, TT], BF16, tag="pqT", name="pqT")
            nc.scalar.activation(pkT, pp2, AF.Identity,
                                 bias=small_cols[("akb", hh)])
            t = keep.tile([DK, TT], BF16, tag=f"kaT{hh}", name=f"kaT{hh}")
            nc.vector.tensor_add(t, kT[hh], pkT)
            kaT[hh] = t

        # masks: row-major per block (abs) and one band-major tile (rel)
        for b in range(BPC):
            for ib, (i0, il) in enumerate(IBLOCKS):
                o = IN_OFF_M + (b * 2 + ib) * T
                t = keep.tile([128, T], F32, tag=f"mb{b}_{ib}",
                              name=f"mb{b}_{ib}")
                nc.vector.tensor_scalar(t[:il, :], inp[:il, o : o + T],
                                        1e9, -1e9, OP.mult, OP.add)
                mb[(b, ib)] = t


    # ---------------- stream / epilogue machinery ----------------
    # Hybrid stream into a row-major [128, 512] psum tile: band 0 (psum
    # partition 0, the only place dual-fp8 matmuls may write) uses paired
    # DoubleRow matmuls (2 rows / 200 cycles); higher bands use fp8-1x
    # per-row matmuls with plane-aware weight slices.  Rows descend
    # within each band so start=True overwrites garbage below.
    def emit_stream(b, ib, hr, s2t, chunks):
        i0, il = IBLOCKS[ib]
        for (ip0, M) in chunks:     # pair-granular chunks
            rkt = rkt_pool.tile([128, 6400], BF16, tag="rkt", name="rkt")
            nc.gpsimd.dma_start(
                rkt[:, : M * 200],
                relk[hr, b][:, i0 // 2 + ip0 : i0 // 2 + ip0 + M, :])
            rkt8 = rkt.bitcast(F8)
            rs_r = rkt8.ap[0][0]
            for p0 in range(0, M, 16):      # 16-pair (32-row) bands
                pl = min(16, M - p0)
                row_base = 2 * (ip0 + p0)
                gp = b * (T // 2) + i0 // 2 + ip0 + p0
                if row_base == 0:
                    for r in range(pl - 1, -1, -1):
                        lhsT = bass.AP(
                            tensor=qW8[hr].tensor,
                            offset=qW8[hr].offset + 2 * gp,
                            ap=[[qW8[hr].ap[0][0], 128], [TT, 2],
                                [1, 2 * (r + 1)]])
                        rhs = bass.AP(
                            tensor=rkt8.tensor,
                            offset=rkt8.offset + (p0 + r) * 400,
                            ap=[[rs_r, 128], [1, 2], [2, T]])
                        nc.tensor.matmul(
                            s2t[0 : 2 * (r + 1), :T],
                            lhsT, rhs,
                            start=True, stop=False,
                            perf_mode=mybir.MatmulPerfMode.DoubleRow,
                            skip_group_check=True,
                            tile_position=(0, 0))
                else:
                    for rr in range(2 * pl - 1, -1, -1):
                        pr, par = rr // 2, rr % 2
                        # row (row_base+rr)'s qW lives in plane `par`; the
                        # other plane's zeros below it are harmless (those
                        # rows are overwritten by later matmuls)
                        q1 = bass.AP(
                            tensor=qW8[hr].tensor,
                            offset=qW8[hr].offset + TT * par + 2 * gp,
                            ap=[[qW8[hr].ap[0][0], 128], [1, rr + 1]])
                        r1 = bass.AP(
                            tensor=rkt8.tensor,
                            offset=rkt8.offset + (p0 + pr) * 400 + par,
                            ap=[[rs_r, 128], [2, T]])
                        nc.tensor.matmul(
                            s2t[row_base : row_base + rr + 1, :T],
                            q1, r1,
                            start=True, stop=False,
                            skip_group_check=True,
                            tile_position=(0, row_base))
    def emit_head_rel(b, ib, h, s2ps, xT_ps):
        """Row-major rel head: st = s2 * 2^-6 + mask -> softmax -> p@v."""
        i0, il = IBLOCKS[ib]
        hr = h - H_ABS
        st = sm.tile([128, T], F32, tag="st", name="st")
        nc.vector.tensor_scalar(st[:il, :], s2ps[hr][:il, :T],
                                1.0 / UP, None, OP.mult)
        nc.vector.tensor_add(st[:il, :], st[:il, :],
                             mb[(b, ib)][:il, :])
        nmax = sm.tile([128, 1], F32, tag="nmax", name="nmax")
        nc.vector.tensor_reduce(nmax[:il], st[:il, :], AX.X, OP.max,
                                negate=True)
        p = sm.tile([128, T], BF16, tag="p", name="p")
        rsum = sm.tile([128, 1], F32, tag="rsum", name="rsum")
        nc.scalar.activation(p[:il, :], st[:il, :], AF.Exp,
                             bias=nmax[:il], accum_out=rsum[:il])
        rcp = sm.tile([128, 1], F32, tag="rcp", name="rcp")
        nc.vector.reciprocal(rcp[:il], rsum[:il])
        nc.vector.tensor_scalar(p[:il, :], p[:il, :], rcp[:il], None,
                                OP.mult)
        hsl = slice(DK * h, DK * (h + 1))
        for jb, (j0, jl) in enumerate(IBLOCKS):
            tp = ps_ty.tile([128, 128], BF16, tag="ty", name="ty")
            nc.tensor.matmul(tp[:jl, :il], p[:il, j0 : j0 + jl],
                             ident[:il, :il], is_transpose=True,
                             skip_group_check=True)
            pT = sm.tile([128, 128], BF16, tag="pT", name="pT")
            nc.scalar.copy(pT[:jl, :il], tp[:jl, :il])
            nc.tensor.matmul(xT_ps[hsl, :il], vb[(b, jb)][:jl, hsl],
                             pT[:jl, :il],
                             start=(jb == 0), stop=(jb == 1),
                             skip_group_check=True,
                             tile_position=(0, DK * h))

    def emit_head_abs(b, ib, h, xT_ps):
        """Row-major path for the absolute-position heads."""
        i0, il = IBLOCKS[ib]
        tsl = slice(b * T + i0, b * T + i0 + il)
        bsl = slice(b * T, (b + 1) * T)
        st = sm.tile([128, T], F32, tag="st", name="st")
        s1 = ps_ty.tile([128, T], F32, tag="ty", name="ty")
        nc.tensor.matmul(s1[:il, :], qaT[h][:, tsl],
                         kaT[h][:, bsl], skip_group_check=True)
        nc.vector.tensor_add(st[:il, :], s1[:il, :],
                             mb[(b, ib)][:il, :])
        nmax = sm.tile([128, 1], F32, tag="nmax", name="nmax")
        nc.vector.tensor_reduce(nmax[:il], st[:il, :], AX.X, OP.max,
                                negate=True)
        p = sm.tile([128, T], BF16, tag="p", name="p")
        rsum = sm.tile([128, 1], F32, tag="rsum", name="rsum")
        nc.scalar.activation(p[:il, :], st[:il, :], AF.Exp,
                             bias=nmax[:il], accum_out=rsum[:il])
        rcp = sm.tile([128, 1], F32, tag="rcp", name="rcp")
        nc.vector.reciprocal(rcp[:il], rsum[:il])
        nc.vector.tensor_scalar(p[:il, :], p[:il, :], rcp[:il], None,
                                OP.mult)
        hsl = slice(DK * h, DK * (h + 1))
        for jb, (j0, jl) in enumerate(IBLOCKS):
            tp = ps_ty.tile([128, 128], BF16, tag="ty", name="ty")
            nc.tensor.matmul(tp[:jl, :il], p[:il, j0 : j0 + jl],
                             ident[:il, :il], is_transpose=True,
                             skip_group_check=True)
            pT = sm.tile([128, 128], BF16, tag="pT", name="pT")
            nc.scalar.copy(pT[:jl, :il], tp[:jl, :il])
            nc.tensor.matmul(xT_ps[hsl, :il], vb[(b, jb)][:jl, hsl],
                             pT[:jl, :il],
                             start=(jb == 0), stop=(jb == 1),
                             skip_group_check=True,
                             tile_position=(0, DK * h))

    def emit_proj(b, ib, xT_ps):
        i0, il = IBLOCKS[ib]
        xT_sb = sm.tile([128, 128], BF16, tag="xT_sb", name="xT_sb")
        nc.scalar.copy(xT_sb[:, :il], xT_ps[:, :il])
        y_ps = ps_ty.tile([128, 128], F32, tag="ty", name="ty")
        nc.tensor.matmul(y_ps[:il, :], xT_sb[:, :il], wo,
                         skip_group_check=True)
        y_sb = keep.tile([128, 128], F32, tag=f"y_out{b}_{ib}",
                         name=f"y_out{b}_{ib}")
        nc.vector.tensor_add(y_sb[:il, :], y_ps[:il, :],
                             reph["bo_b"][:il, :])
        nc.scalar.dma_start(out[b, i0 : i0 + il, :], y_sb[:il, :])

    # ---------------- main schedule ----------------
    emit_rel_qw(0)
    emit_rel_qw(1)
    blocks = [(0, 0), (0, 1), (1, 0), (1, 1)][: BPC * 2]
    for bi, (b, ib) in enumerate(blocks):
        i0, il = IBLOCKS[ib]
        last = bi == len(blocks) - 1
        xT_ps = ps_x.tile([128, 128], F32, tag="xT", name="xT")
        if last:
            # abs heads depend only on prep: run them under the streams
            emit_head_abs(b, ib, 0, xT_ps)
            emit_head_abs(b, ib, 1, xT_ps)
        # pair-granular chunks of the row-block
        npair = il // 2
        chunks = []
        ip = 0
        while ip < npair:
            m = min(16 if (bi == 0 and ip == 0) else 32, npair - ip)
            chunks.append((ip, m))
            ip += m
        s2ps = {}
        for hr in range(H_REL):
            s2ps[hr] = ps_s2.tile([128, 512], F32, tag=f"s2h{hr}",
                                  name=f"s2h{hr}")
            emit_stream(b, ib, hr, s2ps[hr], chunks)
        if bi == 0:
            emit_rest_prep()
        for hr in range(H_REL):
            # content scores (with the folded c row) accumulate on top
            nc.tensor.matmul(s2ps[hr][:il, :T],
                             qT64[hr][: DK + 1,
                                      b * T + i0 : b * T + i0 + il],
                             kT[H_ABS + hr][: DK + 1,
                                            b * T : (b + 1) * T],
                             start=False, stop=True,
                             skip_group_check=True)
        emit_head_rel(b, ib, 2, s2ps, xT_ps)
        emit_head_rel(b, ib, 3, s2ps, xT_ps)
        if not last:
            emit_head_abs(b, ib, 0, xT_ps)
            emit_head_abs(b, ib, 1, xT_ps)
        emit_proj(b, ib, xT_ps)


def build_nc():
    nc = bacc.Bacc(trn_type="TRN2")
    io = {}
    io["inpack"] = nc.dram_tensor(
        "inpack", [128, IN_COLS], BF16, kind="ExternalInput").ap()
    # fp8 byte pairs carried as bf16: [h, b, d, ipair, j]
    io["rel_kernel"] = nc.dram_tensor(
        "rel_kernel", [H_REL, BPC, D, T // 2, T], BF16, kind="ExternalInput"
    ).ap()
    io["out"] = nc.dram_tensor("out", [BPC, T, D], F32,
                               kind="ExternalOutput").ap()

    with tile.TileContext(nc) as tc:
        with ExitStack() as ctx:
            build_kernel(ctx, tc, io)
    nc.compile()
    return nc


_NC_CACHE = None


def _get_nc():
    global _NC_CACHE
    if _NC_CACHE is None:
        _NC_CACHE = build_nc()
    return _NC_CACHE


def make_in_maps(inputs):
    """Shard full inputs into per-core input maps (layout/dtype work only)."""
    import ml_dtypes
    bf = ml_dtypes.bfloat16
    f32 = np.float32
    g = {k: np.asarray(inputs[k], dtype=f32) for k in
         ["Wq", "bq", "Wk", "bk", "Wv", "bv", "abs_q_w", "abs_q_b",
          "abs_k_w", "abs_k_b", "rel_k_w", "rel_k_b", "rel_bias",
          "Wo", "bo"]}
    wp = np.zeros((128, WPACK_COLS), f32)

    def put(nm, arr):
        o = WPACK_OFF[nm]
        arr = np.asarray(arr, f32)
        if arr.ndim == 1:
            arr = arr[:, None]
        wp[: arr.shape[0], o : o + arr.shape[1]] = arr

    put("Wq", g["Wq"]); put("Wk", g["Wk"]); put("Wv", g["Wv"])
    put("Wo", g["Wo"]); put("bq", g["bq"]); put("bk", g["bk"])
    for hr in range(H_REL):
        put(f"rkwT{hr}", g["rel_k_w"][hr].T)  # [32 o, 128 d]
        put(f"rkb{hr}", g["rel_k_b"][hr])
        put(f"rbias{hr}", g["rel_bias"][0, hr, 0, :])
        put(f"bqrb{hr}", g["bq"][DK * (H_ABS + hr) : DK * (H_ABS + hr + 1)]
            + g["rel_bias"][0, hr, 0, :])
    for hh in range(H_ABS):
        put(f"aqw{hh}", g["abs_q_w"][hh])
        put(f"akw{hh}", g["abs_k_w"][hh])
        put(f"aqb{hh}", g["abs_q_b"][hh])
        put(f"akb{hh}", g["abs_k_b"][hh])
    put("bvb", np.tile(g["bv"][None, :], (128, 1)))
    put("bob", np.tile(g["bo"][None, :], (128, 1)))

    query = np.asarray(inputs["query"], dtype=f32)
    key = np.asarray(inputs["key"], dtype=f32)
    value = np.asarray(inputs["value"], dtype=f32)
    mask_i = np.asarray(inputs["mask"], dtype=np.int32)[:, 0]  # [B, T, T]
    absk = np.asarray(inputs["abs_kernel"], dtype=f32)

    # rel_kernel: fp8 e4m3, host-transposed to [h, B, d, pair, j, 2]
    # with byte pair (RK[2m, j, d], RK[2m+1, j, d]) for DoubleRow k-tiles
    rk8 = np.asarray(inputs["rel_kernel"], dtype=f32).astype(
        ml_dtypes.float8_e4m3fn).view(np.uint8)      # [h, B, i, j, d]
    X = np.empty((H_REL, B, D, T // 2, T, 2), np.uint8)
    X[..., 0] = rk8[:, :, 0::2].transpose(0, 1, 4, 2, 3)
    X[..., 1] = rk8[:, :, 1::2].transpose(0, 1, 4, 2, 3)
    relk = X.view(np.uint16).reshape(H_REL, B, D, T // 2, T).view(bf)

    in_maps = []
    for c in range(N_CORES):
        bs = slice(c * BPC, (c + 1) * BPC)
        ip = np.zeros((128, IN_COLS), f32)
        ip[:, :WPACK_COLS] = wp
        ip[:, IN_OFF_ID : IN_OFF_ID + 128] = np.eye(128, dtype=f32)
        ip[:, IN_OFF_Q : IN_OFF_Q + TT] = query[bs].reshape(TT, 128).T
        ip[:, IN_OFF_K : IN_OFF_K + TT] = key[bs].reshape(TT, 128).T
        ip[:, IN_OFF_V : IN_OFF_V + TT] = value[bs].reshape(TT, 128).T
        ip[:, IN_OFF_A0 : IN_OFF_A0 + TT] = absk[0, bs].reshape(TT, 128).T
        ip[:, IN_OFF_A1 : IN_OFF_A1 + TT] = absk[1, bs].reshape(TT, 128).T
        for bl in range(BPC):
            for ib, (i0, il) in enumerate(IBLOCKS):
                o = IN_OFF_M + (bl * 2 + ib) * T
                ip[:il, o : o + T] = mask_i[c * BPC + bl, i0 : i0 + il, :]
        m = {
            "inpack": np.ascontiguousarray(ip.astype(bf)),
            "rel_kernel": np.ascontiguousarray(relk[:, bs]),
        }
        in_maps.append(m)
    return in_maps


def kernel(**inputs) -> np.ndarray:
    nc = _get_nc()
    in_maps = make_in_maps(inputs)
    res = run_bass_kernel_spmd(nc, in_maps, core_ids=list(range(N_CORES)))
    return np.concatenate([r["out"] for r in res.results], axis=0)


if __name__ == "__main__":
    nc = build_nc()
    print("built ok")


# revision 36
# speedup vs baseline: 1.0246x; 1.0246x over previous
"""Trainium2 Bass kernel for nn_Attention_12463995093474 (sparse_attention).

Math (reference):
  q/k/v = content linears; 2 absolute heads, 2 relative heads (DK=32).
  abs:  scores = (Xq_a + abs_kernel@abs_q_w) @ (Xk_a + abs_kernel@abs_k_w)^T
  rel:  scores = Xq_r @ Xk_r^T + (Xq_r + rel_bias) . (rel_kernel@rel_k_w + rel_k_b)
  softmax(mask) @ v -> out linear.

Key algebraic collapse: the dominant rel term
    s2[i,j] = sum_o q''[i,o] * (sum_d RK[i,j,d] W[d,o])  (+ c[i])
            = sum_d qW[i,d] * RK[i,j,d],   qW = q'' @ W^T
so RK contracts DIRECTLY against a per-row vector (32x fewer MACs than
materializing R).  Implementation:
  - rel_kernel is pre-transposed on host to [h, b, d, ipair, j, 2] fp8
    e4m3 (byte pair = rows 2m / 2m+1) so it streams with PLAIN DMAs and
    feeds fp8 dual-row matmuls.
  - per (head, row-PAIR) one PE matmul in MatmulPerfMode.DoubleRow:
    contract 256 = 2 k-tiles of 128 d, k-tile t = row 2m+t's RK; the
    stationary weights select each output row's own k-tile via a
    two-plane layout [qW[even rows] | qW[odd rows]] with zeros in the
    opposite plane.  The moving port runs at 2 B/partition/cycle --
    2x the fp8-1x rate (~43 ns per row of 200 scores).
  - dual-fp8 matmuls must write PSUM partition 0, so only the first
    32-row band of each row-block uses DoubleRow pairs; higher bands use
    fp8-1x per-row matmuls (plane-aware weight slices).  Rows descend
    within each band so start=True overwrites the garbage rows below.
    Content scores (with the c-term folded in as a 33rd contraction row
    of ones x c^T) accumulate on top, then masked softmax -> p@v.
  - qW is scaled by 64 on-device before the fp8 cast (values sigma
    ~0.005 would be subnormal in e4m3); the 2^-6 is folded into the
    epilogue's (s2 + c) * scale DVE op.

Sharding: data-parallel over batch, B=16 -> 2 batches per core on 8 cores.
"""

import numpy as np
from contextlib import ExitStack

import concourse.bass as bass
import concourse.bacc as bacc
import concourse.tile as tile
from concourse import mybir
from concourse.bass_utils import run_bass_kernel_spmd

B, T, D = 16, 200, 128
H_ABS, H_REL, H, DK = 2, 2, 4, 32
N_CORES = 8
BPC = B // N_CORES  # batches per core
SCALE = 1.0 / float(DK) ** 0.5
TT = BPC * T  # tokens per core (400)
UP = 64.0     # qW fp8 upscale (power of 2)

F32 = mybir.dt.float32
BF16 = mybir.dt.bfloat16
F8 = mybir.dt.float8e4
AX = mybir.AxisListType
OP = mybir.AluOpType
AF = mybir.ActivationFunctionType

# i-blocks per batch: (start, len)
IBLOCKS = [(0, 128), (128, T - 128)]


# packed weights, stored [128, col] on host; loaded with one plain DMA
WPACK_LAYOUT = [
    ("Wq", 128, 128), ("Wk", 128, 128), ("Wv", 128, 128), ("Wo", 128, 128),
    ("bq", 128, 1), ("bk", 128, 1),
    ("rkwT0", 32, 128), ("rkwT1", 32, 128),
    ("rkb0", 32, 1), ("rkb1", 32, 1), ("rbias0", 32, 1), ("rbias1", 32, 1),
    ("aqw0", 128, 32), ("aqw1", 128, 32), ("akw0", 128, 32), ("akw1", 128, 32),
    ("aqb0", 32, 1), ("aqb1", 32, 1), ("akb0", 32, 1), ("akb1", 32, 1),
    ("bqrb0", 32, 1), ("bqrb1", 32, 1),
    ("bvb", 128, 128), ("bob", 128, 128),
]
WPACK_OFF = {}
_o = 0
for _nm, _r, _cc in WPACK_LAYOUT:
    WPACK_OFF[_nm] = _o
    _o += _cc
WPACK_COLS = _o
# one host-packed [128, IN_COLS] bf16 tensor
IN_OFF_ID = WPACK_COLS
IN_OFF_Q = IN_OFF_ID + 128
IN_OFF_K = IN_OFF_Q + TT
IN_A_COLS = IN_OFF_K + TT          # part A: weights + ident + xq + xk
IN_OFF_V = IN_A_COLS
IN_OFF_A0 = IN_OFF_V + TT
IN_OFF_A1 = IN_OFF_A0 + TT
IN_OFF_M = IN_OFF_A1 + TT          # row-major mask
IN_COLS = IN_OFF_M + 4 * T


def build_kernel(ctx: ExitStack, tc: tile.TileContext, io: dict):
    nc = tc.nc

    relk = io["rel_kernel"]    # [2, 2, 128, 100, 200] bf16 carrier of fp8
    out = io["out"]            # [2, 200, 128]

    consts = ctx.enter_context(tc.tile_pool(name="consts", bufs=1))
    keep = ctx.enter_context(tc.tile_pool(name="keep", bufs=1))
    prep = ctx.enter_context(tc.tile_pool(name="prep", bufs=2))
    rkt_pool = ctx.enter_context(tc.tile_pool(name="rkt", bufs=7))
    sm = ctx.enter_context(tc.tile_pool(name="sm", bufs=2))
    # NOTE: this (otherwise unused) pool shifts downstream SBUF/semaphore
    # allocation in a way that measurably improves the schedule (~15us);
    # removing it regressed 105us -> 122us on hardware.
    smb = ctx.enter_context(tc.tile_pool(name="smb", bufs=3))
    ps_s2 = ctx.enter_context(tc.tile_pool(name="ps_s2", bufs=2, space="PSUM"))
    ps_pp = ctx.enter_context(tc.tile_pool(name="ps_pp", bufs=1, space="PSUM"))
    ps_x = ctx.enter_context(tc.tile_pool(name="ps_x", bufs=1, space="PSUM"))
    ps_ty = ctx.enter_context(tc.tile_pool(name="ps_ty", bufs=2, space="PSUM"))

    # ---------------- input load (plain DMAs only) ----------------
    # part A (weights+ident+xq+xk) lands first so prep unblocks early.
    inp = consts.tile([128, IN_COLS], BF16, tag="inpack")
    nc.sync.dma_start(inp[:, :IN_A_COLS], io["inpack"][:, :IN_A_COLS])
    nc.scalar.dma_start(inp[:, IN_A_COLS:], io["inpack"][:, IN_A_COLS:])

    def wslice(nm, rows=128):
        o = WPACK_OFF[nm]
        c = dict((n, cc) for n, _r, cc in WPACK_LAYOUT)[nm]
        return inp[:rows, o : o + c]

    xqT = inp[:, IN_OFF_Q : IN_OFF_Q + TT]
    xkT = inp[:, IN_OFF_K : IN_OFF_K + TT]
    xvT = inp[:, IN_OFF_V : IN_OFF_V + TT]
    akT_pre = {0: inp[:, IN_OFF_A0 : IN_OFF_A0 + TT],
               1: inp[:, IN_OFF_A1 : IN_OFF_A1 + TT]}
    ident = inp[:, IN_OFF_ID : IN_OFF_ID + 128]

    wq = wslice("Wq")
    wk = wslice("Wk")
    wv = wslice("Wv")
    wo = wslice("Wo")

    # ---- early consts: what gates the matvec streams + rel content ----
    bqrb64 = {}
    for hr in range(H_REL):
        t = consts.tile([DK, 1], F32, tag=f"bqrb64_{hr}")
        nc.scalar.activation(t, wslice(f"bqrb{hr}", rows=DK), AF.Copy,
                             scale=SCALE * UP)
        bqrb64[hr] = t
    bq_s64 = consts.tile([128, 1], F32, tag="bq_s64")
    nc.scalar.activation(bq_s64, wslice("bq"), AF.Copy, scale=SCALE * UP)
    bk_c = consts.tile([128, 1], F32, tag="bk_c")
    nc.scalar.copy(bk_c, wslice("bk"))

    qrb64 = {}   # rel heads q + rel_bias (x64), bf16 [32, TT]
    qT64 = {}    # rel content lhsT (x64), bf16 [32, TT]
    qW8 = {}     # two-plane DoubleRow weights, fp8 [128, 2*TT]
    kT = {}      # content k, bf16 [32, TT]

    def emit_rel_qw(hr):
        h = H_ABS + hr
        qp = ps_pp.tile([DK, TT], F32, tag="pp", name="pp")
        nc.tensor.matmul(qp, wq[:, DK * h : DK * (h + 1)], xqT,
                         skip_group_check=True)
        t2 = keep.tile([DK, TT], BF16, tag=f"qrb64_{hr}", name=f"qrb64_{hr}")
        nc.scalar.activation(t2, qp, AF.Identity, bias=bqrb64[hr],
                             scale=SCALE * UP)
        qrb64[hr] = t2
        qwp = ps_pp.tile([128, TT], F32, tag="pp", name="pp")
        nc.tensor.matmul(qwp, wslice(f"rkwT{hr}", rows=DK), t2,
                         skip_group_check=True)
        # DoubleRow weights, two-plane layout (ISA: ktile stride must be
        # even + 16B aligned; columns contiguous):
        #   plane0 (cols 0:TT):   qW[row] at even rows, 0 at odd
        #   plane1 (cols TT:2TT): 0 at even rows, qW[row] at odd
        q8 = keep.tile([128, 2 * TT], F8, tag=f"qW8_{hr}", name=f"qW8_{hr}")
        nc.vector.memset(q8, 0.0)
        rs_q = q8.ap[0][0]
        rs_p = qwp.ap[0][0]
        for par in range(2):
            dst = bass.AP(tensor=q8.tensor,
                          offset=q8.offset + TT * par + par,
                          ap=[[rs_q, 128], [2, TT // 2]])
            srcp = bass.AP(tensor=qwp.tensor, offset=qwp.offset + par,
                           ap=[[rs_p, 128], [2, TT // 2]])
            nc.vector.tensor_copy(dst, srcp)
        qW8[hr] = q8
        # rel content operands with the c-term folded in as an extra
        # contraction row: qT64c row 32 = c64^T, kTc row 32 = ones
        t = keep.tile([DK + 1, TT], BF16, tag=f"qT{h}", name=f"qT{h}")
        nc.scalar.activation(t[:DK, :], qp, AF.Identity,
                             bias=bq_s64[DK * h : DK * (h + 1)],
                             scale=SCALE * UP)
        cps = ps_pp.tile([64, TT], F32, tag="pp", name="pp")
        nc.tensor.matmul(cps[DK : DK + 1, :], wslice(f"rkb{hr}", rows=DK),
                         t2, skip_group_check=True,
                         tile_position=(0, DK))
        nc.scalar.copy(t[DK : DK + 1, :], cps[DK : DK + 1, :])
        qT64[hr] = t
        kp = ps_pp.tile([DK, TT], F32, tag="pp", name="pp")
        nc.tensor.matmul(kp, wk[:, DK * h : DK * (h + 1)], xkT,
                         skip_group_check=True)
        tk = keep.tile([DK + 1, TT], BF16, tag=f"kT{h}", name=f"kT{h}")
        nc.scalar.activation(tk[:DK, :], kp, AF.Identity,
                             bias=bk_c[DK * h : DK * (h + 1)])
        nc.vector.memset(tk[DK : DK + 1, :], 1.0)
        kT[h] = tk

    # ---- deferred prep (emitted under the first matvec streams) ----
    qT = {}
    vb = {}
    qaT = {}
    kaT = {}
    mb = {}
    c_sb = {}
    reph = {}

    def emit_rest_prep():
        bq_s = consts.tile([128, 1], F32, tag="bq_s")
        nc.scalar.activation(bq_s, wslice("bq"), AF.Copy, scale=SCALE)
        bv_b = consts.tile([128, 128], F32, tag="bv_b")
        nc.scalar.copy(bv_b, wslice("bvb"))
        bo_b = consts.tile([128, 128], F32, tag="bo_b")
        nc.scalar.copy(bo_b, wslice("bob"))
        reph["bo_b"] = bo_b

        small_cols = {}
        for hh in range(H_ABS):
            ts_ = consts.tile([DK, 1], F32, tag=f"aqb_s{hh}")
            nc.scalar.activation(ts_, wslice(f"aqb{hh}", rows=DK), AF.Copy,
                                 scale=SCALE)
            small_cols[("aqb_s", hh)] = ts_
            akb32 = consts.tile([DK, 1], F32, tag=f"akb32_{hh}")
            nc.scalar.copy(akb32, wslice(f"akb{hh}", rows=DK))
            small_cols[("akb", hh)] = akb32

        for h in range(H_ABS):
            qp = ps_pp.tile([DK, TT], F32, tag="pp", name="pp")
            nc.tensor.matmul(qp, wq[:, DK * h : DK * (h + 1)], xqT,
                             skip_group_check=True)
            t = keep.tile([DK, TT], BF16, tag=f"qT{h}", name=f"qT{h}")
            nc.scalar.activation(t, qp, AF.Identity,
                                 bias=bq_s[DK * h : DK * (h + 1)],
                                 scale=SCALE)
            qT[h] = t
            kp = ps_pp.tile([DK, TT], F32, tag="pp", name="pp")
            nc.tensor.matmul(kp, wk[:, DK * h : DK * (h + 1)], xkT,
                             skip_group_check=True)
            tk = keep.tile([DK, TT], BF16, tag=f"kT{h}", name=f"kT{h}")
            nc.scalar.activation(tk, kp, AF.Identity,
                                 bias=bk_c[DK * h : DK * (h + 1)])
            kT[h] = tk

        for b in range(BPC):
            for jb, (j0, jl) in enumerate(IBLOCKS):
                vp = ps_ty.tile([128, 128], F32, tag="ty", name="ty")
                nc.tensor.matmul(vp[:jl, :],
                                 xvT[:, b * T + j0 : b * T + j0 + jl], wv,
                                 skip_group_check=True)
                t = keep.tile([128, 128], BF16, tag=f"v{b}_{jb}",
                              name=f"v{b}_{jb}")
                nc.vector.tensor_add(t[:jl, :], vp[:jl, :], bv_b[:jl, :])
                vb[(b, jb)] = t

        for hh in range(H_ABS):
            akT = akT_pre[hh]
            pp = ps_pp.tile([DK, TT], F32, tag="pp", name="pp")
            nc.tensor.matmul(pp, wslice(f"aqw{hh}"), akT,
                             skip_group_check=True)
            pqT = prep.tile([DK, TT], BF16, tag="pqT", name="pqT")
            nc.scalar.activation(pqT, pp, AF.Identity,
                                 bias=small_cols[("aqb_s", hh)], scale=SCALE)
            t = keep.tile([DK, TT], BF16, tag=f"qaT{hh}", name=f"qaT{hh}")
            nc.vector.tensor_add(t, qT[hh], pqT)
            qaT[hh] = t

            pp2 = ps_pp.tile([DK, TT], F32, tag="pp", name="pp")
            nc.tensor.matmul(pp2, wslice(f"akw{hh}"), akT,
                             skip_group_check=True)
            pkT = prep.tile([DK, TT], BF16, tag="pqT", name="pqT")
            nc.scalar.activation(pkT, pp2, AF.Identity,
                                 bias=small_cols[("akb", hh)])
            t = keep.tile([DK, TT], BF16, tag=f"kaT{hh}", name=f"kaT{hh}")
            nc.vector.tensor_add(t, kT[hh], pkT)
            kaT[hh] = t

        # masks: row-major per block (abs) and one band-major tile (rel)
        for b in range(BPC):
            for ib, (i0, il) in enumerate(IBLOCKS):
                o = IN_OFF_M + (b * 2 + ib) * T
                t = keep.tile([128, T], F32, tag=f"mb{b}_{ib}",
                              name=f"mb{b}_{ib}")
                nc.vector.tensor_scalar(t[:il, :], inp[:il, o : o + T],
                                        1e9, -1e9, OP.mult, OP.add)
                mb[(b, ib)] = t


    # ---------------- stream / epilogue machinery ----------------
    # Hybrid stream into a row-major [128, 512] psum tile: band 0 (psum
    # partition 0, the only place dual-fp8 matmuls may write) uses paired
    # DoubleRow matmuls (2 rows / 200 cycles); higher bands use fp8-1x
    # per-row matmuls with plane-aware weight slices.  Rows descend
    # within each band so start=True overwrites garbage below.
    def emit_stream(b, ib, hr, s2t, chunks):
        i0, il = IBLOCKS[ib]
        for (ip0, M) in chunks:     # pair-granular chunks
            rkt = rkt_pool.tile([128, 6400], BF16, tag="rkt", name="rkt")
            nc.gpsimd.dma_start(
                rkt[:, : M * 200],
                relk[hr, b][:, i0 // 2 + ip0 : i0 // 2 + ip0 + M, :])
            rkt8 = rkt.bitcast(F8)
            rs_r = rkt8.ap[0][0]
            for p0 in range(0, M, 16):      # 16-pair (32-row) bands
                pl = min(16, M - p0)
                row_base = 2 * (ip0 + p0)
                gp = b * (T // 2) + i0 // 2 + ip0 + p0
                if row_base == 0:
                    for r in range(pl - 1, -1, -1):
                        lhsT = bass.AP(
                            tensor=qW8[hr].tensor,
                            offset=qW8[hr].offset + 2 * gp,
                            ap=[[qW8[hr].ap[0][0], 128], [TT, 2],
                                [1, 2 * (r + 1)]])
                        rhs = bass.AP(
                            tensor=rkt8.tensor,
                            offset=rkt8.offset + (p0 + r) * 400,
                            ap=[[rs_r, 128], [1, 2], [2, T]])
                        nc.tensor.matmul(
                            s2t[0 : 2 * (r + 1), :T],
                            lhsT, rhs,
                            start=True, stop=False,
                            perf_mode=mybir.MatmulPerfMode.DoubleRow,
                            skip_group_check=True,
                            tile_position=(0, 0))
                else:
                    for rr in range(2 * pl - 1, -1, -1):
                        pr, par = rr // 2, rr % 2
                        # row (row_base+rr)'s qW lives in plane `par`; the
                        # other plane's zeros below it are harmless (those
                        # rows are overwritten by later matmuls)
                        q1 = bass.AP(
                            tensor=qW8[hr].tensor,
                            offset=qW8[hr].offset + TT * par + 2 * gp,
                            ap=[[qW8[hr].ap[0][0], 128], [1, rr + 1]])
                        r1 = bass.AP(
                            tensor=rkt8.tensor,
                            offset=rkt8.offset + (p0 + pr) * 400 + par,
                            ap=[[rs_r, 128], [2, T]])
                        nc.tensor.matmul(
                            s2t[row_base : row_base + rr + 1, :T],
                            q1, r1,
                            start=True, stop=False,
                            skip_group_check=True,
                            tile_position=(0, row_base))
    def emit_head_rel(b, ib, h, s2ps, xT_ps):
        """Row-major rel head: st = s2 * 2^-6 + mask -> softmax -> p@v."""
        i0, il = IBLOCKS[ib]
        hr = h - H_ABS
        st = sm.tile([128, T], F32, tag="st", name="st")
        nc.vector.tensor_scalar(st[:il, :], s2ps[hr][:il, :T],
                                1.0 / UP, None, OP.mult)
        nc.vector.tensor_add(st[:il, :], st[:il, :],
                             mb[(b, ib)][:il, :])
        nmax = sm.tile([128, 1], F32, tag="nmax", name="nmax")
        nc.vector.tensor_reduce(nmax[:il], st[:il, :], AX.X, OP.max,
                                negate=True)
        p = sm.tile([128, T], BF16, tag="p", name="p")
        rsum = sm.tile([128, 1], F32, tag="rsum", name="rsum")
        nc.scalar.activation(p[:il, :], st[:il, :], AF.Exp,
                             bias=nmax[:il], accum_out=rsum[:il])
        rcp = sm.tile([128, 1], F32, tag="rcp", name="rcp")
        nc.vector.reciprocal(rcp[:il], rsum[:il])
        nc.vector.tensor_scalar(p[:il, :], p[:il, :], rcp[:il], None,
                                OP.mult)
        hsl = slice(DK * h, DK * (h + 1))
        for jb, (j0, jl) in enumerate(IBLOCKS):
            tp = ps_ty.tile([128, 128], BF16, tag="ty", name="ty")
            nc.tensor.matmul(tp[:jl, :il], p[:il, j0 : j0 + jl],
                             ident[:il, :il], is_transpose=True,
                             skip_group_check=True)
            pT = sm.tile([128, 128], BF16, tag="pT", name="pT")
            nc.scalar.copy(pT[:jl, :il], tp[:jl, :il])
            nc.tensor.matmul(xT_ps[hsl, :il], vb[(b, jb)][:jl, hsl],
                             pT[:jl, :il],
                             start=(jb == 0), stop=(jb == 1),
                             skip_group_check=True,
                             tile_position=(0, DK * h))

    def emit_head_abs(b, ib, h, xT_ps):
        """Row-major path for the absolute-position heads."""
        i0, il = IBLOCKS[ib]
        tsl = slice(b * T + i0, b * T + i0 + il)
        bsl = slice(b * T, (b + 1) * T)
        st = sm.tile([128, T], F32, tag="st", name="st")
        s1 = ps_ty.tile([128, T], F32, tag="ty", name="ty")
        nc.tensor.matmul(s1[:il, :], qaT[h][:, tsl],
                         kaT[h][:, bsl], skip_group_check=True)
        nc.vector.tensor_add(st[:il, :], s1[:il, :],
                             mb[(b, ib)][:il, :])
        nmax = sm.tile([128, 1], F32, tag="nmax", name="nmax")
        nc.vector.tensor_reduce(nmax[:il], st[:il, :], AX.X, OP.max,
                                negate=True)
        p = sm.tile([128, T], BF16, tag="p", name="p")
        rsum = sm.tile([128, 1], F32, tag="rsum", name="rsum")
        nc.scalar.activation(p[:il, :], st[:il, :], AF.Exp,
                             bias=nmax[:il], accum_out=rsum[:il])
        rcp = sm.tile([128, 1], F32, tag="rcp", name="rcp")
        nc.vector.reciprocal(rcp[:il], rsum[:il])
        nc.vector.tensor_scalar(p[:il, :], p[:il, :], rcp[:il], None,
                                OP.mult)
        hsl = slice(DK * h, DK * (h + 1))
        for jb, (j0, jl) in enumerate(IBLOCKS):
            tp = ps_ty.tile([128, 128], BF16, tag="ty", name="ty")
            nc.tensor.matmul(tp[:jl, :il], p[:il, j0 : j0 + jl],
                             ident[:il, :il], is_transpose=True,
                             skip_group_check=True)
            pT = sm.tile([128, 128], BF16, tag="pT", name="pT")
            nc.scalar.copy(pT[:jl, :il], tp[:jl, :il])
            nc.tensor.matmul(xT_ps[hsl, :il], vb[(b, jb)][:jl, hsl],
                             pT[:jl, :il],
                             start=(jb == 0), stop=(jb == 1),
                             skip_group_check=True,
                             tile_position=(0, DK * h))

    def emit_proj(b, ib, xT_ps):
        i0, il = IBLOCKS[ib]
        xT_sb = sm.tile([128, 128], BF16, tag="xT_sb", name="xT_sb")
        nc.scalar.copy(xT_sb[:, :il], xT_ps[:, :il])
        y_ps = ps_ty.tile([128, 128], F32, tag="ty", name="ty")
        nc.tensor.matmul(y_ps[:il, :], xT_sb[:, :il], wo,
                         skip_group_check=True)
        y_sb = keep.tile([128, 128], F32, tag=f"y_out{b}_{ib}",
                         name=f"y_out{b}_{ib}")
        nc.vector.tensor_add(y_sb[:il, :], y_ps[:il, :],
                             reph["bo_b"][:il, :])
        nc.scalar.dma_start(out[b, i0 : i0 + il, :], y_sb[:il, :])

    # ---------------- main schedule ----------------
    emit_rel_qw(0)
    emit_rel_qw(1)
    blocks = [(0, 0), (0, 1), (1, 0), (1, 1)][: BPC * 2]
    for bi, (b, ib) in enumerate(blocks):
        i0, il = IBLOCKS[ib]
        last = bi == len(blocks) - 1
        xT_ps = ps_x.tile([128, 128], F32, tag="xT", name="xT")
        if last:
            # abs heads depend only on prep: run them under the streams
            emit_head_abs(b, ib, 0, xT_ps)
            emit_head_abs(b, ib, 1, xT_ps)
        # pair-granular chunks of the row-block
        npair = il // 2
        chunks = []
        ip = 0
        while ip < npair:
            m = min(16 if (bi == 0 and ip == 0) else 32, npair - ip)
            chunks.append((ip, m))
            ip += m
        s2ps = {}
        for hr in range(H_REL):
            s2ps[hr] = ps_s2.tile([128, 512], F32, tag=f"s2h{hr}",
                                  name=f"s2h{hr}")
            emit_stream(b, ib, hr, s2ps[hr], chunks)
        if bi == 0:
            emit_rest_prep()
        for hr in range(H_REL):
            # content scores (with the folded c row) accumulate on top
            nc.tensor.matmul(s2ps[hr][:il, :T],
                             qT64[hr][: DK + 1,
                                      b * T + i0 : b * T + i0 + il],
                             kT[H_ABS + hr][: DK + 1,
                                            b * T : (b + 1) * T],
                             start=False, stop=True,
                             skip_group_check=True)
        emit_head_rel(b, ib, 2, s2ps, xT_ps)
        emit_head_rel(b, ib, 3, s2ps, xT_ps)
        if not last:
            emit_head_abs(b, ib, 0, xT_ps)
            emit_head_abs(b, ib, 1, xT_ps)
        emit_proj(b, ib, xT_ps)


def build_nc():
    nc = bacc.Bacc(trn_type="TRN2")
    io = {}
    io["inpack"] = nc.dram_tensor(
        "inpack", [128, IN_COLS], BF16, kind="ExternalInput").ap()
    # fp8 byte pairs carried as bf16: [h, b, d, ipair, j]
    io["rel_kernel"] = nc.dram_tensor(
        "rel_kernel", [H_REL, BPC, D, T // 2, T], BF16, kind="ExternalInput"
    ).ap()
    io["out"] = nc.dram_tensor("out", [BPC, T, D], F32,
                               kind="ExternalOutput").ap()

    with tile.TileContext(nc) as tc:
        with ExitStack() as ctx:
            build_kernel(ctx, tc, io)
    nc.compile()
    return nc


_NC_CACHE = None


def _get_nc():
    global _NC_CACHE
    if _NC_CACHE is None:
        _NC_CACHE = build_nc()
    return _NC_CACHE


def make_in_maps(inputs):
    """Shard full inputs into per-core input maps (layout/dtype work only)."""
    import ml_dtypes
    bf = ml_dtypes.bfloat16
    f32 = np.float32
    g = {k: np.asarray(inputs[k], dtype=f32) for k in
         ["Wq", "bq", "Wk", "bk", "Wv", "bv", "abs_q_w", "abs_q_b",
          "abs_k_w", "abs_k_b", "rel_k_w", "rel_k_b", "rel_bias",
          "Wo", "bo"]}
    wp = np.zeros((128, WPACK_COLS), f32)

    def put(nm, arr):
        o = WPACK_OFF[nm]
        arr = np.asarray(arr, f32)
        if arr.ndim == 1:
            arr = arr[:, None]
        wp[: arr.shape[0], o : o + arr.shape[1]] = arr

    put("Wq", g["Wq"]); put("Wk", g["Wk"]); put("Wv", g["Wv"])
    put("Wo", g["Wo"]); put("bq", g["bq"]); put("bk", g["bk"])
    for hr in range(H_REL):
        put(f"rkwT{hr}", g["rel_k_w"][hr].T)  # [32 o, 128 d]
        put(f"rkb{hr}", g["rel_k_b"][hr])
        put(f"rbias{hr}", g["rel_bias"][0, hr, 0, :])
        put(f"bqrb{hr}", g["bq"][DK * (H_ABS + hr) : DK * (H_ABS + hr + 1)]
            + g["rel_bias"][0, hr, 0, :])
    for hh in range(H_ABS):
        put(f"aqw{hh}", g["abs_q_w"][hh])
        put(f"akw{hh}", g["abs_k_w"][hh])
        put(f"aqb{hh}", g["abs_q_b"][hh])
        put(f"akb{hh}", g["abs_k_b"][hh])
    put("bvb", np.tile(g["bv"][None, :], (128, 1)))
    put("bob", np.tile(g["bo"][None, :], (128, 1)))

    query = np.asarray(inputs["query"], dtype=f32)
    key = np.asarray(inputs["key"], dtype=f32)
    value = np.asarray(inputs["value"], dtype=f32)
    mask_i = np.asarray(inputs["mask"], dtype=np.int32)[:, 0]  # [B, T, T]
    absk = np.asarray(inputs["abs_kernel"], dtype=f32)

    # rel_kernel: fp8 e4m3, host-transposed to [h, B, d, pair, j, 2]
    # with byte pair (RK[2m, j, d], RK[2m+1, j, d]) for DoubleRow k-tiles
    rk8 = np.asarray(inputs["rel_kernel"], dtype=f32).astype(
        ml_dtypes.float8_e4m3fn).view(np.uint8)      # [h, B, i, j, d]
    X = np.empty((H_REL, B, D, T // 2, T, 2), np.uint8)
    X[..., 0] = rk8[:, :, 0::2].transpose(0, 1, 4, 2, 3)
    X[..., 1] = rk8[:, :, 1::2].transpose(0, 1, 4, 2, 3)
    relk = X.view(np.uint16).reshape(H_REL, B, D, T // 2, T).view(bf)

    in_maps = []
    for c in range(N_CORES):
        bs = slice(c * BPC, (c + 1) * BPC)
        ip = np.zeros((128, IN_COLS), f32)
        ip[:, :WPACK_COLS] = wp
        ip[:, IN_OFF_ID : IN_OFF_ID + 128] = np.eye(128, dtype=f32)
        ip[:, IN_OFF_Q : IN_OFF_Q + TT] = query[bs].reshape(TT, 128).T
        ip[:, IN_OFF_K : IN_OFF_K + TT] = key[bs].reshape(TT, 128).T
        ip[:, IN_OFF_V : IN_OFF_V + TT] = value[bs].reshape(TT, 128).T
        ip[:, IN_OFF_A0 : IN_OFF_A0 + TT] = absk[0, bs].reshape(TT, 128).T
        ip[:, IN_OFF_A1 : IN_OFF_A1 + TT] = absk[1, bs].reshape(TT, 128).T
        for bl in range(BPC):
            for ib, (i0, il) in enumerate(IBLOCKS):
                o = IN_OFF_M + (bl * 2 + ib) * T
                ip[:il, o : o + T] = mask_i[c * BPC + bl, i0 : i0 + il, :]
        m = {
            "inpack": np.ascontiguousarray(ip.astype(bf)),
            "rel_kernel": np.ascontiguousarray(relk[:, bs]),
        }
        in_maps.append(m)
    return in_maps


def kernel(**inputs) -> np.ndarray:
    nc = _get_nc()
    in_maps = make_in_maps(inputs)
    res = run_bass_kernel_spmd(nc, in_maps, core_ids=list(range(N_CORES)))
    return np.concatenate([r["out"] for r in res.results], axis=0)


if __name__ == "__main__":
    nc = build_nc()
    print("built ok")
